# revision 1
# baseline (speedup 1.0000x reference)
"""Multi-head attention (B=4, S=2048, D=1024, H=16) on 8 trn2 NeuronCores.

Sharding: tensor-parallel over heads — core c owns heads [2c, 2c+1]
(= model dims [128c, 128c+128)).  Each core computes q/k/v projections for
its head slice (full batch), local attention, and a partial out-projection
against its 128 columns of Wo.  The 8 partial [B*S, D] outputs are summed
on the host (the all-reduce of the Megatron pattern, done at gather time).

Per-core kernel (bf16 matmul operands, fp32 PSUM accumulation):
  xT  [D, B*S]   : x transposed on host so the contraction dim lands on
                   SBUF partitions (avoids on-chip transposes of x).
  qT/kT [128, S] per batch : head-major [2*64, tokens].
  scores_T [k, q] in PSUM -> exp via ScalarE (scale=1/8 fused) -> bf16.
  v_ext [k-tile, 65*2]     : v natural layout (PE-transposed) with a ones
                   column per head => attn@v yields numerator + softmax
                   denominator in one pass.
  division: fast-NR reciprocal of the denom row (fp32), K=1 fp32r matmul
                   broadcast across partitions, DVE multiply.
  out-proj: outT [128, S] (head dims on partitions) @ WoT slice.

Scheduling: Tile's per-engine issue order follows emission order, and PSUM
accumulation groups MUST be contiguous on the PE (interleaving any other
matmul corrupts them / NRT_EXEC_UNIT_UNRECOVERABLE).  The attention exp is
ScalarE-paced, so scores phases leave the PE under-occupied and the HAM
clock-gate re-throttles it to 1.2 GHz.  To keep the PE dense we hand-
interleave self-contained PE work units (projection groups, v-transposes,
attn@v groups of the previous q-chunk, divisions) between the score
matmuls of the current q-chunk — each unit internally contiguous.
"""
import os
import sys

sys.path.insert(0, "/opt/trn_rl_repo")

import numpy as np

import concourse.mybir as mybir
import concourse.tile as tile
from concourse import bacc
from concourse._compat import with_exitstack
from concourse.bass_utils import run_bass_kernel_spmd
from concourse.masks import make_identity
from contextlib import ExitStack

B, S, D, H = 4, 2048, 1024, 16
HD = D // H              # 64
P = 128
NCORES = 8
NH = H // NCORES         # 2 heads per core
T = B * S                # 8192 tokens
DT = D // P              # 8 contraction tiles
KT = S // P              # 16 k-tiles per batch
QC = 1024                # q-chunk (2 psum banks, halves ACT overhead)
NQC = S // QC            # 2
HQ = 512                 # matmul free-dim chunk (one psum bank)
TC = 512                 # projection token chunk
NTC = S // TC            # 4
EXP_SCALE = float(1.0 / np.sqrt(HD))

f32 = mybir.dt.float32
f32r = mybir.dt.float32r
bf16 = mybir.dt.bfloat16

LAST_EXEC_TIME_NS = None
_CACHED_NC = None


@with_exitstack
def _mha_kernel(ctx: ExitStack, tc_: tile.TileContext, ins, outs):
    nc = tc_.nc
    xT_d, wqT_d, wkT_d, wvT_d, woT_d, ones_d = ins
    out_d = outs[0]

    const = ctx.enter_context(tc_.tile_pool(name="const", bufs=1))
    wpool = ctx.enter_context(tc_.tile_pool(name="wpool", bufs=1))
    xpool = ctx.enter_context(tc_.tile_pool(name="xpool", bufs=3))
    qpool = ctx.enter_context(tc_.tile_pool(name="qpool", bufs=1))
    kpool = ctx.enter_context(tc_.tile_pool(name="kpool", bufs=1))
    vpool = ctx.enter_context(tc_.tile_pool(name="vpool", bufs=1))
    vxpool = ctx.enter_context(tc_.tile_pool(name="vxpool", bufs=1))
    epool = ctx.enter_context(tc_.tile_pool(name="epool", bufs=48))
    opool = ctx.enter_context(tc_.tile_pool(name="opool", bufs=1))
    rpool = ctx.enter_context(tc_.tile_pool(name="rpool", bufs=2))
    ospool = ctx.enter_context(tc_.tile_pool(name="ospool", bufs=2))

    # single PSUM pool: 4 rotating [128,1024] slots (2 banks each)
    psum = ctx.enter_context(tc_.tile_pool(name="psum", bufs=4, space="PSUM"))

    ident = const.tile([P, P], f32, tag="ident")
    make_identity(nc, ident)
    ones_b = const.tile([P, 2], bf16, tag="ones_b")
    nc.gpsimd.dma_start(ones_b[:], ones_d[:, 0:2])
    ones_r = const.tile([1, HD], f32r, tag="ones_r")
    nc.sync.dma_start(ones_r[:], ones_d[0:1, 2:2 + HD].bitcast(f32r))

    wq = wpool.tile([P, D], bf16, tag="wq")
    wk = wpool.tile([P, D], bf16, tag="wk")
    wv = wpool.tile([P, D], bf16, tag="wv")
    wo = wpool.tile([P, D], bf16, tag="wo")

    xT_r = xT_d[:].rearrange("(dt p) t -> p dt t", p=P)

    xts = {}

    def load_x(t, lb):
        xt = xpool.tile([P, DT * TC], bf16, tag="xt", name=f"xt_{lb}_{t}")
        nc.gpsimd.dma_start(
            xt[:].rearrange("p (dt t) -> p dt t", dt=DT),
            xT_r[:, :, lb * S + t * TC: lb * S + (t + 1) * TC],
        )
        xts[(lb, t)] = xt

    def load_w(w_sb, w_d):
        # [D, 128] DRAM -> [128, DT*128] SBUF, d-tile major, cast to bf16
        nc.gpsimd.dma_start(
            w_sb[:].rearrange("p (dt o) -> p dt o", dt=DT),
            w_d[:].rearrange("(dt p) o -> p dt o", p=P),
        )

    # first projection (k, chunk 0) only needs wk + x0: ship those first
    load_w(wk, wkT_d)
    load_x(0, 0)
    load_w(wq, wqT_d)
    load_x(1, 0)
    load_w(wv, wvT_d)
    nc.gpsimd.dma_start(wo[:], woT_d[:])

    carry = []   # deferred out-projections of the previous batch

    for b in range(B):
        qT = qpool.tile([P, S], bf16, tag="qT")
        kT = kpool.tile([P, S], bf16, tag="kT")
        vT = vpool.tile([P, S], f32, tag="vT")
        v_ext = vxpool.tile([P, KT * 2 * (HD + 1)], bf16, tag="vext")
        outT = opool.tile([P, S], bf16, tag="outT")


        def proj(w_sb, dstT, t):
            # one contiguous 8-matmul accumulation group + eviction
            pp = psum.tile([P, QC], f32, tag="ps")
            for dt in range(DT):
                nc.tensor.matmul(
                    pp[:, 0:TC],
                    w_sb[:, dt * P:(dt + 1) * P],
                    xts[(b, t)][:, dt * TC:(dt + 1) * TC],
                    start=(dt == 0), stop=(dt == DT - 1),
                )
            nc.vector.tensor_copy(dstT[:, t * TC:(t + 1) * TC], pp[:, 0:TC])

        def vtrans(kt):
            vps = psum.tile([P, QC], f32, tag="ps")
            nc.tensor.transpose(
                vps[:, 0:P], vT[:, kt * P:(kt + 1) * P], ident[:])
            base = kt * 2 * (HD + 1)
            nc.vector.tensor_copy(v_ext[:, base:base + HD], vps[:, 0:HD])
            nc.vector.tensor_copy(
                v_ext[:, base + HD + 1:base + 2 * HD + 1], vps[:, HD:2 * HD])
            nc.vector.tensor_copy(v_ext[:, base + HD:base + HD + 1],
                                  ones_b[:, 0:1])
            nc.vector.tensor_copy(
                v_ext[:, base + 2 * HD + 1:base + 2 * HD + 2], ones_b[:, 1:2])

        def scores(qc, kt, exp_tiles):
            for h in range(NH):
                hs = slice(h * HD, (h + 1) * HD)
                sc = psum.tile([P, QC], f32, tag="ps")
                for half in range(QC // HQ):
                    nc.tensor.matmul(
                        sc[:, half * HQ:(half + 1) * HQ],
                        kT[hs, kt * P:(kt + 1) * P],
                        qT[hs, qc * QC + half * HQ: qc * QC + (half + 1) * HQ],
                        start=True, stop=True,
                    )
                ex = epool.tile([P, QC], bf16, tag="exp")
                nc.scalar.activation(
                    ex[:], sc[:], mybir.ActivationFunctionType.Exp,
                    scale=EXP_SCALE)
                exp_tiles[h][kt] = ex

        def attnv_group(oes, h, half, exp_tiles):
            # one contiguous 16-matmul accumulation group
            if oes[h] is None:
                oes[h] = psum.tile([P, QC], f32, tag="ps", name=f"oe_h{h}")
            hq = slice(half * HQ, (half + 1) * HQ)
            for kt in range(KT):
                base = kt * 2 * (HD + 1) + h * (HD + 1)
                nc.tensor.matmul(
                    oes[h][0:HD + 1, hq],
                    v_ext[:, base:base + HD + 1],
                    exp_tiles[h][kt][:, hq],
                    start=(kt == 0), stop=(kt == KT - 1),
                )

        def div_pre(oes, h, recrs):
            # DVE-only reciprocal chain; emit as early as possible
            oe = oes[h]
            den = rpool.tile([1, QC], f32, tag="den")
            # custom-DVE ops mis-read partition-offset inputs: stage the
            # denominator row to partition 0 first
            nc.vector.tensor_copy(den[:], oe[HD:HD + 1, :])
            rec = rpool.tile([1, QC], f32, tag="rec")
            scr = rpool.tile([1, QC], f32, tag="scr")
            nc.vector.reciprocal_approx_accurate(rec[:], den[:], scr[:])
            recr = rpool.tile([1, QC], f32r, tag="recr")
            with nc.allow_low_precision(reason="feeds bcast matmul"):
                nc.vector.tensor_copy(recr[:], rec[:])
            recrs[h] = recr

        def div_post(oes, qc, h, recrs):
            oe = oes[h]
            qs = slice(qc * QC, (qc + 1) * QC)
            recr = recrs[h]
            rb_ps = psum.tile([P, QC], f32, tag="ps")
            for half in range(QC // HQ):
                hq = slice(half * HQ, (half + 1) * HQ)
                nc.tensor.matmul(rb_ps[0:HD, hq], ones_r[:], recr[:, hq],
                                 start=True, stop=True)
            rb = rpool.tile([HD, QC], f32, tag="rb")
            nc.vector.tensor_copy(rb[:], rb_ps[0:HD, :])
            nc.vector.tensor_mul(outT[h * HD:(h + 1) * HD, qs],
                                 oe[0:HD, :], rb[:])

        def oproj(t, oT=outT, bb=b):
            # oT/bb bound at def time: carried closures must keep THIS
            # batch's outT and row base, not the next iteration's
            po = psum.tile([P, QC], f32, tag="ps")
            for ec in range(D // HQ):
                nc.tensor.matmul(
                    po[:, ec * HQ:(ec + 1) * HQ],
                    oT[:, t * P:(t + 1) * P],
                    wo[:, ec * HQ:(ec + 1) * HQ],
                    start=True, stop=True,
                )
            osb = ospool.tile([P, D], f32, tag="osb")
            nc.vector.tensor_copy(osb[:], po[:])
            nc.sync.dma_start(
                out_d[bb * S + t * P: bb * S + (t + 1) * P, :],
                osb[:],
            )

        # ---------- batch schedule ----------
        # prologue: x chunks 0-1, k/q/v projections 0-1, v-transposes 0-7,
        # interleaved with the previous batch's deferred out-projections
        if (b, 0) not in xts:
            load_x(0, b)
        if (b, 1) not in xts:
            load_x(1, b)
        prologue = []
        for t in (0, 1):
            prologue += [
                (lambda tt: lambda: proj(wk, kT, tt))(t),
                (lambda tt: lambda: proj(wq, qT, tt))(t),
                (lambda tt: lambda: proj(wv, vT, tt))(t),
            ]
        prologue.append(lambda: load_x(2, b))
        prologue.append(lambda: (vtrans(0), vtrans(1), vtrans(2), vtrans(3)))
        prologue.append(lambda: (vtrans(4), vtrans(5), vtrans(6), vtrans(7)))
        for u in prologue:
            u()
            if carry:
                carry.pop(0)()
        while carry:
            carry.pop(0)()

        exp0 = [[None] * KT for _ in range(NH)]
        # qc0 scores with the remaining projections/transposes as PE filler
        fillers = [
            lambda: proj(wk, kT, 2),
            lambda: (load_x(3, b), proj(wk, kT, 3)),
            lambda: proj(wq, qT, 2),
            lambda: proj(wv, vT, 2),
            lambda: (vtrans(8), vtrans(9)),
            lambda: proj(wq, qT, 3),
            lambda: proj(wv, vT, 3),
            lambda: (vtrans(10), vtrans(11)),
            lambda: (vtrans(12), vtrans(13)),
            lambda: (vtrans(14), vtrans(15)),
        ]
        n0 = len(fillers)
        done0 = 0
        for kt in range(KT):
            scores(0, kt, exp0)
            want = (kt + 1) * n0 // KT
            while done0 < want and fillers:
                fillers.pop(0)()
                done0 += 1

        # qc1 scores with qc0's attn@v groups + divisions as PE filler
        exp1 = [[None] * KT for _ in range(NH)]
        oes0 = [None, None]
        recrs0 = [None, None]
        fillers = [
            lambda: attnv_group(oes0, 0, 0, exp0),
            lambda: attnv_group(oes0, 0, 1, exp0),
            lambda: (div_pre(oes0, 0, recrs0), attnv_group(oes0, 1, 0, exp0)),
            lambda: div_post(oes0, 0, 0, recrs0),
            lambda: (attnv_group(oes0, 1, 1, exp0),
                     div_pre(oes0, 1, recrs0),
                     load_x(0, b + 1) if b + 1 < B else None),
            lambda: (div_post(oes0, 0, 1, recrs0),
                     load_x(1, b + 1) if b + 1 < B else None),
        ] + [(lambda tt: lambda: oproj(tt))(t_) for t_ in range(6)]
        n1 = len(fillers)
        done1 = 0
        for kt in range(KT):
            scores(1, kt, exp1)
            want = (kt + 1) * n1 // KT
            while done1 < want and fillers:
                fillers.pop(0)()
                done1 += 1
        for f in fillers:
            f()

        # flush qc1 attn@v + divisions, interleaved with out-projection
        oes1 = [None, None]
        recrs1 = [None, None]
        oproj(6)
        attnv_group(oes1, 0, 0, exp1)
        oproj(7)
        attnv_group(oes1, 0, 1, exp1)
        div_pre(oes1, 0, recrs1)
        attnv_group(oes1, 1, 0, exp1)
        div_post(oes1, 1, 0, recrs1)
        attnv_group(oes1, 1, 1, exp1)
        div_pre(oes1, 1, recrs1)
        div_post(oes1, 1, 1, recrs1)
        carry = [(lambda tt, op: lambda: op(tt))(t_, oproj)
                 for t_ in range(8, S // P)]
        if b == B - 1:
            while carry:
                carry.pop(0)()


def _build():
    global _CACHED_NC
    if _CACHED_NC is not None:
        return _CACHED_NC
    nc = bacc.Bacc("TRN2", target_bir_lowering=False, debug=False)
    xT = nc.dram_tensor("xT", [D, T], f32, kind="ExternalInput").ap()
    wqT = nc.dram_tensor("wqT", [D, P], f32, kind="ExternalInput").ap()
    wkT = nc.dram_tensor("wkT", [D, P], f32, kind="ExternalInput").ap()
    wvT = nc.dram_tensor("wvT", [D, P], f32, kind="ExternalInput").ap()
    woT = nc.dram_tensor("woT", [P, D], f32, kind="ExternalInput").ap()
    ones = nc.dram_tensor("ones", [P, HD + 2], f32, kind="ExternalInput").ap()
    out = nc.dram_tensor("out", [T, D], f32, kind="ExternalOutput").ap()

    with tile.TileContext(nc) as tc_:
        _mha_kernel(tc_, [xT, wqT, wkT, wvT, woT, ones], [out])
    nc.compile()
    _CACHED_NC = nc
    return nc


def kernel(x: np.ndarray, Wq: np.ndarray, Wk: np.ndarray, Wv: np.ndarray,
           Wo: np.ndarray) -> np.ndarray:
    global LAST_EXEC_TIME_NS
    nc = _build()

    x = np.asarray(x, dtype=np.float32)
    xT = np.ascontiguousarray(x.reshape(T, D).T)          # [D, T]
    ones_in = np.ones((P, HD + 2), dtype=np.float32)

    in_maps = []
    for c in range(NCORES):
        rows = slice(c * P, (c + 1) * P)
        in_maps.append({
            "xT": xT,
            "wqT": np.ascontiguousarray(np.asarray(Wq, np.float32)[rows, :].T),
            "wkT": np.ascontiguousarray(np.asarray(Wk, np.float32)[rows, :].T),
            "wvT": np.ascontiguousarray(np.asarray(Wv, np.float32)[rows, :].T),
            "woT": np.ascontiguousarray(np.asarray(Wo, np.float32)[:, rows].T),
            "ones": ones_in,
        })

    trace = bool(os.environ.get("BASS_TRACE"))
    res = run_bass_kernel_spmd(nc, in_maps, core_ids=list(range(NCORES)),
                               trace=trace)
    LAST_EXEC_TIME_NS = res.exec_time_ns

    acc = res.results[0]["out"].astype(np.float32)
    for c in range(1, NCORES):
        acc = acc + res.results[c]["out"]
    return acc.reshape(B, S, D)



# revision 2
# speedup vs baseline: 1.3137x; 1.3137x over previous
"""Multi-head attention (B=4, S=2048, D=1024, H=16) on 8 trn2 NeuronCores.

Sharding (v2): data-parallel over batch x 2-way tensor-parallel over heads.
Core c owns batch b = c//2 and heads [8*(c%2), 8*(c%2)+8) (= model dims
[512*(c%2), 512*(c%2)+512)).  Each core: q/k/v projections for its 8 heads
over its batch's 2048 tokens, attention, partial out-projection against its
512 columns of Wo.  Host sums the 2 partial outputs per batch (all-reduce
of the Megatron pattern at gather time).  vs 8-way head-TP this cuts
per-core HBM traffic 4x (4.2MB bf16 in + 8.4MB f32 out).

Per-core kernel:
  - Heads processed in 4 PAIRS.  Score matmuls have contraction = HD = 64,
    so each kt's two heads run as ROW-TILED CONCURRENT matmuls (PE row
    groups 0-63 / 64-127, auto-derived from base partitions) into separate
    PSUM tiles -> ~2x score throughput vs serial 64-contraction matmuls.
  - exp on ScalarE per (kt, head) [128,1024] tile; ACTIVATEs pipeline at
    ~(N+171)/1.2ns, so this costs only ~7% over 2048-wide tiles while
    halving PSUM (2 banks/tile).
  - v projection in FLIPPED layout (lhsT = x chunk, rhs = Wv): v lands
    [tokens, dims] directly -- zero PE transposes.  A ones column per head
    in v_ext makes attn@v emit numerator + softmax denominator together.
  - attn@v per (head, qc-half, kt-half): 8-matmul groups into one rotating
    PSUM bank, DVE-evicted/accumulated into SBUF f32.  The kt-split lets
    the first half run inside its own stretch, halving exp liveness.
  - division: DVE fast-reciprocal of the denominator row, GPSIMD
    partition_broadcast to 64 rows, DVE multiply into outT (bf16).
  - out-proj contracts all 512 head dims (all pairs) -> runs late; f32
    [2048, 1024] partials out, host adds core pairs.

PSUM (8 banks): scores 3x[128,1024] (6) + attnv 1x[128,512] + shared
proj/oproj 1x[128,512].  An emission-order scheduler pumps an urgent queue
(attnv/divisions -- free exp tiles + PSUM) and a background queue
(projections, out-proj) between score/exp emissions to keep the PE dense
(HAM stays warm) while ScalarE paces the pipeline.
"""
import os
import sys

sys.path.insert(0, "/opt/trn_rl_repo")

from collections import deque
from contextlib import ExitStack

import numpy as np
import ml_dtypes

import concourse.mybir as mybir
import concourse.tile as tile
from concourse import bacc
from concourse._compat import with_exitstack
from concourse.bass_utils import run_bass_kernel_spmd

B, S, D, H = 4, 2048, 1024, 16
HD = D // H              # 64
P = 128
NCORES = 8
ET = D // P              # 8 contraction e-tiles
NPAIR = 4                # head pairs per core (8 heads)
KT = S // P              # 16 key tiles
QC = 1024                # query chunk (stretch granularity)
NQC = S // QC            # 2
TC = 512                 # q/k projection token chunk
VW = 8 * (HD + 1)        # v_ext cols per kt = 520
EXP_SCALE = float(1.0 / np.sqrt(HD))
EPOOL_BUFS = 30

f32 = mybir.dt.float32
bf16 = mybir.dt.bfloat16
Exp = mybir.ActivationFunctionType.Exp

LAST_EXEC_TIME_NS = None
_CACHED_NC = None


@with_exitstack
def _mha_kernel(ctx: ExitStack, tc_: tile.TileContext, ins, outs):
    nc = tc_.nc
    xt_d, wq_d, wk_d, wv_d, wo_d = ins
    out_d = outs[0]

    xpool = ctx.enter_context(tc_.tile_pool(name="xpool", bufs=1))
    wpool = ctx.enter_context(tc_.tile_pool(name="wpool", bufs=1))
    qkpool = ctx.enter_context(tc_.tile_pool(name="qkpool", bufs=2))
    vxpool = ctx.enter_context(tc_.tile_pool(name="vxpool", bufs=1))
    opool = ctx.enter_context(tc_.tile_pool(name="opool", bufs=1))
    ocpool = ctx.enter_context(tc_.tile_pool(name="ocpool", bufs=2))
    dpool = ctx.enter_context(tc_.tile_pool(name="dpool", bufs=2))
    rbpool = ctx.enter_context(tc_.tile_pool(name="rbpool", bufs=2))
    ospool = ctx.enter_context(tc_.tile_pool(name="ospool", bufs=2))
    epool = ctx.enter_context(tc_.tile_pool(name="epool", bufs=EPOOL_BUFS))

    scp = ctx.enter_context(tc_.tile_pool(name="scp", bufs=3, space="PSUM"))
    oep = ctx.enter_context(tc_.tile_pool(name="oep", bufs=1, space="PSUM"))
    mpp = ctx.enter_context(tc_.tile_pool(name="mpp", bufs=1, space="PSUM"))

    # flat 2D layouts; host pre-arranges to match
    xt = xpool.tile([P, ET * S], bf16, tag="xt")            # [e-tile, tok]
    wq = wpool.tile([P, ET * NPAIR * P], bf16, tag="wq")    # [et, pair, hd]
    wk = wpool.tile([P, ET * NPAIR * P], bf16, tag="wk")
    wv = wpool.tile([P, ET * 512], bf16, tag="wvo")         # slot reused by wo
    v_ext = vxpool.tile([P, KT * VW], bf16, tag="vx")       # [kt, 8h, 65]
    outT = opool.tile([P, NPAIR * S], bf16, tag="outT")     # [hd-tile, tok]

    nc.gpsimd.dma_start(wk[:], wk_d[:])
    nc.gpsimd.dma_start(xt[:], xt_d[:])
    nc.sync.dma_start(wq[:], wq_d[:])
    nc.sync.dma_start(wv[:], wv_d[:])

    # ones columns of v_ext (col 64 of each head block), set once
    vcols = v_ext[:].rearrange("p (kh c) -> p kh c", c=HD + 1)
    nc.vector.memset(vcols[:, :, HD:HD + 1], 1.0)

    qTs, kTs = {}, {}
    exps = {}
    oecps, denss = {}, {}
    wo_box = {}

    # ---------------- unit bodies ----------------
    def alloc_qk(p):
        if p not in kTs:
            qTs[p] = qkpool.tile([P, S], bf16, tag="qT", name=f"qT{p}")
            kTs[p] = qkpool.tile([P, S], bf16, tag="kT", name=f"kT{p}")

    def proj_qk(w, dst, p, c):
        # one 8-matmul accumulation group: [128 pair-dims, 512 tokens]
        pp = mpp.tile([P, TC], f32, tag="mp")
        for et in range(ET):
            nc.tensor.matmul(
                pp[:],
                w[:, (et * NPAIR + p) * P:(et * NPAIR + p + 1) * P],
                xt[:, et * S + c * TC: et * S + (c + 1) * TC],
                start=(et == 0), stop=(et == ET - 1),
            )
        nc.vector.tensor_copy(dst[:, c * TC:(c + 1) * TC], pp[:])

    def proj_v(c):
        # flipped: [128 tokens of kt-tile c, 512 v-dims]
        pv = mpp.tile([P, TC], f32, tag="mp")
        for et in range(ET):
            nc.tensor.matmul(
                pv[:],
                xt[:, et * S + c * P: et * S + (c + 1) * P],
                wv[:, et * 512:(et + 1) * 512],
                start=(et == 0), stop=(et == ET - 1),
            )
        dst = v_ext[:, c * VW:(c + 1) * VW].rearrange(
            "p (h c2) -> p h c2", c2=HD + 1)[:, :, 0:HD]
        nc.vector.tensor_copy(dst, pv[:].rearrange("p (h c2) -> p h c2", c2=HD))

    def load_wo():
        wo = wpool.tile([P, NPAIR * D], bf16, tag="wvo", name="wo")
        nc.sync.dma_start(wo[:], wo_d[:])
        wo_box["wo"] = wo

    def scores_unit(p, qc, kt):
        # row-tiled concurrent head pair: h0 rows 0-63, h1 rows 64-127
        sc0 = scp.tile([P, QC], f32, tag="sc", name=f"sc{p}_{qc}_{kt}_0")
        sc1 = scp.tile([P, QC], f32, tag="sc", name=f"sc{p}_{qc}_{kt}_1")
        kTp, qTp = kTs[p], qTs[p]
        for l in range(2):
            for h, sc in ((0, sc0), (1, sc1)):
                rows = slice(h * HD, (h + 1) * HD)
                nc.tensor.matmul(
                    sc[:, l * 512:(l + 1) * 512],
                    kTp[rows, kt * P:(kt + 1) * P],
                    qTp[rows, qc * QC + l * 512: qc * QC + (l + 1) * 512],
                    start=True, stop=True,
                )
        for h, sc in ((0, sc0), (1, sc1)):
            ex = epool.tile([P, QC], bf16, tag="exp", name=f"ex{p}_{qc}_{kt}_{h}")
            nc.scalar.activation(ex[:], sc[:], Exp, scale=EXP_SCALE)
            exps[(p, qc, kt, h)] = ex

    def attnv_unit(p, qc, h, l, kh):
        # one contiguous 8-matmul accumulation group over kt half kh
        key = (p, qc, h)
        if key not in oecps:
            oecps[key] = ocpool.tile([P, QC], f32, tag="ocp",
                                     name=f"ocp{p}_{qc}_{h}")
            denss[key] = dpool.tile([1, QC], f32, tag="dens",
                                    name=f"den{p}_{qc}_{h}")
        oe = oep.tile([P, 512], f32, tag="oe")
        base = (2 * p + h) * (HD + 1)
        for i in range(8):
            kt = kh * 8 + i
            nc.tensor.matmul(
                oe[0:HD + 1, :],
                v_ext[:, kt * VW + base: kt * VW + base + HD + 1],
                exps[(p, qc, kt, h)][:, l * 512:(l + 1) * 512],
                start=(i == 0), stop=(i == 7),
            )
        ocp, dn = oecps[key], denss[key]
        ls = slice(l * 512, (l + 1) * 512)
        if kh == 0:
            nc.vector.tensor_copy(ocp[0:HD, ls], oe[0:HD, :])
            nc.vector.tensor_copy(dn[0:1, ls], oe[HD:HD + 1, :])
        else:
            nc.vector.tensor_add(ocp[0:HD, ls], ocp[0:HD, ls], oe[0:HD, :])
            nc.vector.tensor_add(dn[0:1, ls], dn[0:1, ls], oe[HD:HD + 1, :])

    def div_unit(p, qc, h):
        key = (p, qc, h)
        rec = dpool.tile([1, QC], f32, tag="recs", name=f"rec{p}_{qc}_{h}")
        nc.vector.reciprocal_approx_fast(rec[:], denss[key][:])
        rb = rbpool.tile([HD, QC], f32, tag="rb", name=f"rb{p}_{qc}_{h}")
        nc.gpsimd.partition_broadcast(rb[:], rec[:])
        dst = outT[h * HD:(h + 1) * HD, p * S + qc * QC: p * S + (qc + 1) * QC]
        nc.vector.tensor_mul(dst, oecps[key][0:HD, :], rb[:])

    def oproj_unit(qc, tc):
        t0 = (qc * 8 + tc) * P
        wo = wo_box["wo"]
        osb = ospool.tile([P, D], f32, tag="osb")
        for eh in range(2):
            po = mpp.tile([P, TC], f32, tag="mp")
            for ht in range(NPAIR):
                nc.tensor.matmul(
                    po[:],
                    outT[:, ht * S + t0: ht * S + t0 + P],
                    wo[:, ht * D + eh * 512: ht * D + (eh + 1) * 512],
                    start=(ht == 0), stop=(ht == NPAIR - 1),
                )
            nc.vector.tensor_copy(osb[:, eh * 512:(eh + 1) * 512], po[:])
        nc.sync.dma_start(out_d[t0:t0 + P, :], osb[:])

    # ---------------- scheduler ----------------
    urgent = deque()   # (fn, cycles)
    backg = deque()    # (fn, cycles, ready_si, tag)
    cur_si = 0
    budget = 0.0

    def pump(room):
        nonlocal budget
        budget = min(budget + room, 10000.0)
        while budget > 0:
            if urgent:
                fn, cyc = urgent.popleft()
            elif backg and backg[0][2] <= cur_si:
                fn, cyc, _, _ = backg.popleft()
            else:
                break
            fn()
            budget -= cyc

    def pump_until(tag):
        # emit queued units in order until no `tag` units remain in backg
        while any(t == tag for _, _, _, t in backg):
            if urgent:
                fn, cyc = urgent.popleft()
            else:
                fn, cyc, _, _ = backg.popleft()
            fn()

    # ---------------- lead-in ----------------
    alloc_qk(0)
    for c in range(4):
        proj_qk(wk, kTs[0], 0, c)
    proj_qk(wq, qTs[0], 0, 0)
    proj_qk(wq, qTs[0], 0, 1)

    backg.append((lambda: proj_qk(wq, qTs[0], 0, 2), 4400, 0, "kq0"))
    backg.append((lambda: proj_qk(wq, qTs[0], 0, 3), 4400, 0, "kq0"))
    for c in range(KT):
        backg.append(((lambda cc: lambda: proj_v(cc))(c), 4400, 0, "pv"))
    backg.append((load_wo, 100, 0, "wo"))
    for p in range(1, NPAIR):
        ready = max(0, 2 * p - 2)
        for c in range(4):
            backg.append((
                (lambda pp, cc: lambda: (alloc_qk(pp),
                                         proj_qk(wk, kTs[pp], pp, cc))[-1])(p, c),
                4400, ready, f"kq{p}"))
        for c in range(4):
            backg.append((
                (lambda pp, cc: lambda: proj_qk(wq, qTs[pp], pp, cc))(p, c),
                4400, ready, f"kq{p}"))

    # ---------------- stretches ----------------
    stretches = [(p, qc) for p in range(NPAIR) for qc in range(NQC)]
    for si, (p, qc) in enumerate(stretches):
        cur_si = si
        if qc == 0 and p > 0:
            pump_until(f"kq{p}")   # scores(p) need qT/kT(p) emitted first
        for kt in range(KT):
            scores_unit(p, qc, kt)
            if kt == 7:
                if si == 0:
                    pump_until("pv")   # attnv needs v_ext complete
                for h in range(2):
                    for l in range(2):
                        urgent.append((
                            (lambda a, b, c2, d: lambda: attnv_unit(a, b, c2, d, 0)
                             )(p, qc, h, l), 4400))
            pump(3700)
        # second kt-halves + divisions, consumed during the next stretch
        for h in range(2):
            for l in range(2):
                urgent.append((
                    (lambda a, b, c2, d: lambda: attnv_unit(a, b, c2, d, 1)
                     )(p, qc, h, l), 4400))
            urgent.append((
                (lambda a, b, c2: lambda: div_unit(a, b, c2))(p, qc, h), 600))
        if p == NPAIR - 1:
            # out-proj for this qc: available once p3's divisions (just
            # queued ahead of these in-order) have been emitted
            for tc in range(8):
                backg.append((
                    (lambda q2, t2: lambda: oproj_unit(q2, t2))(qc, tc),
                    4800, si, "po"))

    cur_si = len(stretches)
    while urgent or backg:
        pump(10000)


def _build():
    global _CACHED_NC
    if _CACHED_NC is not None:
        return _CACHED_NC
    nc = bacc.Bacc("TRN2", target_bir_lowering=False, debug=False)
    xt = nc.dram_tensor("xt", [P, ET * S], bf16, kind="ExternalInput").ap()
    wq = nc.dram_tensor("wq", [P, ET * NPAIR * P], bf16,
                        kind="ExternalInput").ap()
    wk = nc.dram_tensor("wk", [P, ET * NPAIR * P], bf16,
                        kind="ExternalInput").ap()
    wv = nc.dram_tensor("wv", [P, ET * 512], bf16, kind="ExternalInput").ap()
    wo = nc.dram_tensor("wo", [P, NPAIR * D], bf16, kind="ExternalInput").ap()
    out = nc.dram_tensor("out", [S, D], f32, kind="ExternalOutput").ap()

    with tile.TileContext(nc) as tc_:
        _mha_kernel(tc_, [xt, wq, wk, wv, wo], [out])
    nc.compile()
    _CACHED_NC = nc
    return nc


def kernel(x: np.ndarray, Wq: np.ndarray, Wk: np.ndarray, Wv: np.ndarray,
           Wo: np.ndarray) -> np.ndarray:
    global LAST_EXEC_TIME_NS
    nc = _build()
    bf = ml_dtypes.bfloat16

    x = np.asarray(x, dtype=np.float32)
    Wq = np.asarray(Wq, np.float32)
    Wk = np.asarray(Wk, np.float32)
    Wv = np.asarray(Wv, np.float32)
    Wo = np.asarray(Wo, np.float32)

    in_maps = []
    for c in range(NCORES):
        b, tp = c // 2, c % 2
        hs = tp * 512
        # xt: [D, S] -> [et, 128, S] -> [128, et*S]
        xt = np.ascontiguousarray(
            x[b].T.reshape(ET, P, S).transpose(1, 0, 2)).astype(bf)
        # wq/wk: W[hs:hs+512, :].T = [e, hd] -> [et, 128, pair, 128] -> p-first
        wq = np.ascontiguousarray(
            Wq[hs:hs + 512, :].T.reshape(ET, P, NPAIR, P)
            .transpose(1, 0, 2, 3)).astype(bf)
        wk = np.ascontiguousarray(
            Wk[hs:hs + 512, :].T.reshape(ET, P, NPAIR, P)
            .transpose(1, 0, 2, 3)).astype(bf)
        wv = np.ascontiguousarray(
            Wv[hs:hs + 512, :].T.reshape(ET, P, 512)
            .transpose(1, 0, 2)).astype(bf)
        # wo: Wo[:, hs:hs+512].T = [hd, e] -> [hdtile, 128, 1024] -> p-first
        wo = np.ascontiguousarray(
            Wo[:, hs:hs + 512].T.reshape(NPAIR, P, D)
            .transpose(1, 0, 2)).astype(bf)
        in_maps.append({
            "xt": xt.reshape(P, ET * S),
            "wq": wq.reshape(P, ET * NPAIR * P),
            "wk": wk.reshape(P, ET * NPAIR * P),
            "wv": wv.reshape(P, ET * 512),
            "wo": wo.reshape(P, NPAIR * D),
        })

    trace = bool(os.environ.get("BASS_TRACE"))
    res = run_bass_kernel_spmd(nc, in_maps, core_ids=list(range(NCORES)),
                               trace=trace)
    LAST_EXEC_TIME_NS = res.exec_time_ns

    outs = [np.asarray(r["out"], np.float32) for r in res.results]
    return np.stack([outs[2 * b] + outs[2 * b + 1] for b in range(B)])


# revision 3
# speedup vs baseline: 1.3832x; 1.0529x over previous
"""Multi-head attention (B=4, S=2048, D=1024, H=16) on 8 trn2 NeuronCores.

Sharding (v2): data-parallel over batch x 2-way tensor-parallel over heads.
Core c owns batch b = c//2 and heads [8*(c%2), 8*(c%2)+8) (= model dims
[512*(c%2), 512*(c%2)+512)).  Each core: q/k/v projections for its 8 heads
over its batch's 2048 tokens, attention, partial out-projection against its
512 columns of Wo.  Host sums the 2 partial outputs per batch (all-reduce
of the Megatron pattern at gather time).  vs 8-way head-TP this cuts
per-core HBM traffic 4x (4.2MB bf16 in + 8.4MB f32 out).

Per-core kernel:
  - Heads processed in 4 PAIRS.  Score matmuls have contraction = HD = 64,
    so each kt's two heads run as ROW-TILED CONCURRENT matmuls (PE row
    groups 0-63 / 64-127, auto-derived from base partitions) into separate
    PSUM tiles -> ~2x score throughput vs serial 64-contraction matmuls.
  - exp on ScalarE per (kt, head) [128,1024] tile; ACTIVATEs pipeline at
    ~(N+171)/1.2ns, so this costs only ~7% over 2048-wide tiles while
    halving PSUM (2 banks/tile).
  - v projection in FLIPPED layout (lhsT = x chunk, rhs = Wv): v lands
    [tokens, dims] directly -- zero PE transposes.  A ones column per head
    in v_ext makes attn@v emit numerator + softmax denominator together.
  - attn@v per (head, qc-half, kt-half): 8-matmul groups into one rotating
    PSUM bank, DVE-evicted/accumulated into SBUF f32.  The kt-split lets
    the first half run inside its own stretch, halving exp liveness.
  - division: DVE fast-reciprocal of the denominator row, GPSIMD
    partition_broadcast to 64 rows, DVE multiply into outT (bf16).
  - out-proj contracts all 512 head dims (all pairs) -> runs late; f32
    [2048, 1024] partials out, host adds core pairs.

PSUM (8 banks): scores 3x[128,1024] (6) + attnv 1x[128,512] + shared
proj/oproj 1x[128,512].  An emission-order scheduler pumps an urgent queue
(attnv/divisions -- free exp tiles + PSUM) and a background queue
(projections, out-proj) between score/exp emissions to keep the PE dense
(HAM stays warm) while ScalarE paces the pipeline.
"""
import os
import sys

sys.path.insert(0, "/opt/trn_rl_repo")

from collections import deque
from contextlib import ExitStack

import numpy as np
import ml_dtypes

import concourse.mybir as mybir
import concourse.tile as tile
from concourse import bacc
from concourse._compat import with_exitstack
from concourse.bass_utils import run_bass_kernel_spmd

B, S, D, H = 4, 2048, 1024, 16
HD = D // H              # 64
P = 128
NCORES = 8
ET = D // P              # 8 contraction e-tiles
NPAIR = 4                # head pairs per core (8 heads)
KT = S // P              # 16 key tiles
QC = 1024                # query chunk (stretch granularity)
NQC = S // QC            # 2
TC = 512                 # q/k projection token chunk
VW = 8 * (HD + 1)        # v_ext cols per kt = 520
EXP_SCALE = float(1.0 / np.sqrt(HD))
EPOOL_BUFS = 30

f32 = mybir.dt.float32
bf16 = mybir.dt.bfloat16
Exp = mybir.ActivationFunctionType.Exp

LAST_EXEC_TIME_NS = None
_CACHED_NC = None


@with_exitstack
def _mha_kernel(ctx: ExitStack, tc_: tile.TileContext, ins, outs):
    nc = tc_.nc
    xt_d, wq_d, wk_d, wv_d, wo_d = ins
    out_d = outs[0]

    xpool = ctx.enter_context(tc_.tile_pool(name="xpool", bufs=1))
    wpool = ctx.enter_context(tc_.tile_pool(name="wpool", bufs=1))
    qkpool = ctx.enter_context(tc_.tile_pool(name="qkpool", bufs=2))
    vxpool = ctx.enter_context(tc_.tile_pool(name="vxpool", bufs=1))
    opool = ctx.enter_context(tc_.tile_pool(name="opool", bufs=1))
    ocpool = ctx.enter_context(tc_.tile_pool(name="ocpool", bufs=2))
    dpool = ctx.enter_context(tc_.tile_pool(name="dpool", bufs=2))
    rbpool = ctx.enter_context(tc_.tile_pool(name="rbpool", bufs=2))
    ospool = ctx.enter_context(tc_.tile_pool(name="ospool", bufs=2))
    epool = ctx.enter_context(tc_.tile_pool(name="epool", bufs=EPOOL_BUFS))

    scp = ctx.enter_context(tc_.tile_pool(name="scp", bufs=3, space="PSUM"))
    oep = ctx.enter_context(tc_.tile_pool(name="oep", bufs=1, space="PSUM"))
    mpp = ctx.enter_context(tc_.tile_pool(name="mpp", bufs=1, space="PSUM"))

    # flat 2D layouts; host pre-arranges to match
    xt = xpool.tile([P, ET * S], bf16, tag="xt")            # [e-tile, tok]
    wq = wpool.tile([P, ET * NPAIR * P], bf16, tag="wq")    # [et, pair, hd]
    wk = wpool.tile([P, ET * NPAIR * P], bf16, tag="wk")
    wv = wpool.tile([P, ET * 512], bf16, tag="wvo")         # slot reused by wo
    v_ext = vxpool.tile([P, KT * VW], bf16, tag="vx")       # [kt, 8h, 65]
    outT = opool.tile([P, NPAIR * S], bf16, tag="outT")     # [hd-tile, tok]

    nc.gpsimd.dma_start(wk[:], wk_d[:])
    # x streamed in 4 token-chunks so PK(c) can start as soon as chunk c
    # lands: xt layout [128, et, tok] -> chunk c covers tok [c*512,(c+1)*512)
    xt3 = xt[:].rearrange("p (e t) -> p e t", e=ET)
    xd3 = xt_d[:].rearrange("p (e t) -> p e t", e=ET)
    for c in range(4):
        nc.gpsimd.dma_start(xt3[:, :, c * TC:(c + 1) * TC],
                            xd3[:, :, c * TC:(c + 1) * TC])
    nc.sync.dma_start(wq[:], wq_d[:])
    nc.sync.dma_start(wv[:], wv_d[:])

    # ones columns of v_ext (col 64 of each head block), set once
    vcols = v_ext[:].rearrange("p (kh c) -> p kh c", c=HD + 1)
    nc.vector.memset(vcols[:, :, HD:HD + 1], 1.0)

    qTs, kTs = {}, {}
    exps = {}
    oecps, denss = {}, {}
    wo_box = {}

    # ---------------- unit bodies ----------------
    def alloc_qk(p):
        if p not in kTs:
            qTs[p] = qkpool.tile([P, S], bf16, tag="qT", name=f"qT{p}")
            kTs[p] = qkpool.tile([P, S], bf16, tag="kT", name=f"kT{p}")

    def proj_qk(w, dst, p, c):
        # one 8-matmul accumulation group: [128 pair-dims, 512 tokens]
        pp = mpp.tile([P, TC], f32, tag="mp")
        for et in range(ET):
            nc.tensor.matmul(
                pp[:],
                w[:, (et * NPAIR + p) * P:(et * NPAIR + p + 1) * P],
                xt[:, et * S + c * TC: et * S + (c + 1) * TC],
                start=(et == 0), stop=(et == ET - 1),
            )
        nc.vector.tensor_copy(dst[:, c * TC:(c + 1) * TC], pp[:])

    def proj_v(c):
        # flipped: [128 tokens of kt-tile c, 512 v-dims]
        pv = mpp.tile([P, TC], f32, tag="mp")
        for et in range(ET):
            nc.tensor.matmul(
                pv[:],
                xt[:, et * S + c * P: et * S + (c + 1) * P],
                wv[:, et * 512:(et + 1) * 512],
                start=(et == 0), stop=(et == ET - 1),
            )
        dst = v_ext[:, c * VW:(c + 1) * VW].rearrange(
            "p (h c2) -> p h c2", c2=HD + 1)[:, :, 0:HD]
        nc.vector.tensor_copy(dst, pv[:].rearrange("p (h c2) -> p h c2", c2=HD))

    def load_wo():
        wo = wpool.tile([P, NPAIR * D], bf16, tag="wvo", name="wo")
        nc.sync.dma_start(wo[:], wo_d[:])
        wo_box["wo"] = wo

    def scores_unit(p, qc, kt):
        # row-tiled concurrent head pair: h0 rows 0-63, h1 rows 64-127
        sc0 = scp.tile([P, QC], f32, tag="sc", name=f"sc{p}_{qc}_{kt}_0")
        sc1 = scp.tile([P, QC], f32, tag="sc", name=f"sc{p}_{qc}_{kt}_1")
        kTp, qTp = kTs[p], qTs[p]
        for l in range(2):
            for h, sc in ((0, sc0), (1, sc1)):
                rows = slice(h * HD, (h + 1) * HD)
                nc.tensor.matmul(
                    sc[:, l * 512:(l + 1) * 512],
                    kTp[rows, kt * P:(kt + 1) * P],
                    qTp[rows, qc * QC + l * 512: qc * QC + (l + 1) * 512],
                    start=True, stop=True,
                )
        for h, sc in ((0, sc0), (1, sc1)):
            ex = epool.tile([P, QC], bf16, tag="exp", name=f"ex{p}_{qc}_{kt}_{h}")
            nc.scalar.activation(ex[:], sc[:], Exp, scale=EXP_SCALE)
            exps[(p, qc, kt, h)] = ex

    def attnv_unit(p, qc, h, l, kh):
        # one contiguous 8-matmul accumulation group over kt half kh
        key = (p, qc, h)
        if key not in oecps:
            oecps[key] = ocpool.tile([P, QC], f32, tag="ocp",
                                     name=f"ocp{p}_{qc}_{h}")
            denss[key] = dpool.tile([1, QC], f32, tag="dens",
                                    name=f"den{p}_{qc}_{h}")
        oe = oep.tile([P, 512], f32, tag="oe")
        base = (2 * p + h) * (HD + 1)
        for i in range(8):
            kt = kh * 8 + i
            nc.tensor.matmul(
                oe[0:HD + 1, :],
                v_ext[:, kt * VW + base: kt * VW + base + HD + 1],
                exps[(p, qc, kt, h)][:, l * 512:(l + 1) * 512],
                start=(i == 0), stop=(i == 7),
            )
        ocp, dn = oecps[key], denss[key]
        ls = slice(l * 512, (l + 1) * 512)
        if kh == 0:
            nc.vector.tensor_copy(ocp[0:HD, ls], oe[0:HD, :])
            nc.vector.tensor_copy(dn[0:1, ls], oe[HD:HD + 1, :])
        else:
            nc.vector.tensor_add(ocp[0:HD, ls], ocp[0:HD, ls], oe[0:HD, :])
            nc.vector.tensor_add(dn[0:1, ls], dn[0:1, ls], oe[HD:HD + 1, :])

    def div_unit(p, qc, h):
        key = (p, qc, h)
        rec = dpool.tile([1, QC], f32, tag="recs", name=f"rec{p}_{qc}_{h}")
        nc.vector.reciprocal_approx_fast(rec[:], denss[key][:])
        rb = rbpool.tile([HD, QC], f32, tag="rb", name=f"rb{p}_{qc}_{h}")
        nc.gpsimd.partition_broadcast(rb[:], rec[:])
        dst = outT[h * HD:(h + 1) * HD, p * S + qc * QC: p * S + (qc + 1) * QC]
        nc.vector.tensor_mul(dst, oecps[key][0:HD, :], rb[:])

    in_drain = [False]

    def oproj_unit(qc, tc):
        t0 = (qc * 8 + tc) * P
        wo = wo_box["wo"]
        osb = ospool.tile([P, D], f32, tag="osb")
        for eh in range(2):
            po = (scp.tile([P, QC], f32, tag="sc", name=f"po{qc}_{tc}_{eh}")
                  if in_drain[0] else mpp.tile([P, TC], f32, tag="mp"))
            for ht in range(NPAIR):
                nc.tensor.matmul(
                    po[0:P, 0:TC],
                    outT[:, ht * S + t0: ht * S + t0 + P],
                    wo[:, ht * D + eh * 512: ht * D + (eh + 1) * 512],
                    start=(ht == 0), stop=(ht == NPAIR - 1),
                )
            nc.vector.tensor_copy(osb[:, eh * 512:(eh + 1) * 512],
                                  po[0:P, 0:TC])
        nc.sync.dma_start(out_d[t0:t0 + P, :], osb[:])

    # ---------------- scheduler ----------------
    urgent = deque()   # (fn, cycles)
    backg = deque()    # (fn, cycles, ready_si, tag)
    cur_si = 0
    budget = 0.0

    def pump(room):
        nonlocal budget
        budget = min(budget + room, 6000.0)
        while budget > 0:
            if urgent:
                fn, cyc = urgent.popleft()
            elif backg and backg[0][2] <= cur_si:
                fn, cyc, _, _ = backg.popleft()
            else:
                break
            fn()
            budget -= cyc

    def pump_until(tag):
        # emit queued units in order until no `tag` units remain in backg
        while any(t == tag for _, _, _, t in backg):
            if urgent:
                fn, cyc = urgent.popleft()
            else:
                fn, cyc, _, _ = backg.popleft()
            fn()

    # ---------------- lead-in ----------------
    alloc_qk(0)
    for c in range(4):
        proj_qk(wk, kTs[0], 0, c)
        if c < 2:
            proj_qk(wq, qTs[0], 0, c)

    backg.append((lambda: proj_qk(wq, qTs[0], 0, 2), 4400, 0, "kq0"))
    backg.append((lambda: proj_qk(wq, qTs[0], 0, 3), 4400, 0, "kq0"))
    for c in range(KT):
        backg.append(((lambda cc: lambda: proj_v(cc))(c), 4400, 0, "pv"))
    backg.append((load_wo, 100, 0, "wo"))
    for p in range(1, NPAIR):
        ready = max(0, 2 * p - 2)
        for c in range(4):
            backg.append((
                (lambda pp, cc: lambda: (alloc_qk(pp),
                                         proj_qk(wk, kTs[pp], pp, cc))[-1])(p, c),
                4400, ready, f"kq{p}"))
        for c in range(4):
            backg.append((
                (lambda pp, cc: lambda: proj_qk(wq, qTs[pp], pp, cc))(p, c),
                4400, ready, f"kq{p}"))

    # ---------------- stretches ----------------
    stretches = [(p, qc) for p in range(NPAIR) for qc in range(NQC)]
    for si, (p, qc) in enumerate(stretches):
        cur_si = si
        if qc == 0 and p > 0:
            pump_until(f"kq{p}")   # scores(p) need qT/kT(p) emitted first
        for kt in range(KT):
            scores_unit(p, qc, kt)
            if kt == 7 and si == 0:
                pump_until("pv")   # attnv needs v_ext complete
            if kt in (7, 9, 11, 13):
                h, l = divmod((kt - 7) // 2, 2)
                urgent.append((
                    (lambda a, b, c2, d: lambda: attnv_unit(a, b, c2, d, 0)
                     )(p, qc, h, l), 4400))
            pump(4600)
        # second kt-halves + divisions, consumed during the next stretch
        for h in range(2):
            for l in range(2):
                urgent.append((
                    (lambda a, b, c2, d: lambda: attnv_unit(a, b, c2, d, 1)
                     )(p, qc, h, l), 4400))
            urgent.append((
                (lambda a, b, c2: lambda: div_unit(a, b, c2))(p, qc, h), 600))
        if p == NPAIR - 1:
            # out-proj for this qc: available once p3's divisions (just
            # queued ahead of these in-order) have been emitted
            for tc in range(8):
                backg.append((
                    (lambda q2, t2: lambda: oproj_unit(q2, t2))(qc, tc),
                    4800, si, "po"))

    cur_si = len(stretches)
    in_drain[0] = True
    while urgent or backg:
        pump(10000)


def _build():
    global _CACHED_NC
    if _CACHED_NC is not None:
        return _CACHED_NC
    nc = bacc.Bacc("TRN2", target_bir_lowering=False, debug=False)
    xt = nc.dram_tensor("xt", [P, ET * S], bf16, kind="ExternalInput").ap()
    wq = nc.dram_tensor("wq", [P, ET * NPAIR * P], bf16,
                        kind="ExternalInput").ap()
    wk = nc.dram_tensor("wk", [P, ET * NPAIR * P], bf16,
                        kind="ExternalInput").ap()
    wv = nc.dram_tensor("wv", [P, ET * 512], bf16, kind="ExternalInput").ap()
    wo = nc.dram_tensor("wo", [P, NPAIR * D], bf16, kind="ExternalInput").ap()
    out = nc.dram_tensor("out", [S, D], f32, kind="ExternalOutput").ap()

    with tile.TileContext(nc) as tc_:
        _mha_kernel(tc_, [xt, wq, wk, wv, wo], [out])
    nc.compile()
    _CACHED_NC = nc
    return nc


def kernel(x: np.ndarray, Wq: np.ndarray, Wk: np.ndarray, Wv: np.ndarray,
           Wo: np.ndarray) -> np.ndarray:
    global LAST_EXEC_TIME_NS
    nc = _build()
    bf = ml_dtypes.bfloat16

    x = np.asarray(x, dtype=np.float32)
    Wq = np.asarray(Wq, np.float32)
    Wk = np.asarray(Wk, np.float32)
    Wv = np.asarray(Wv, np.float32)
    Wo = np.asarray(Wo, np.float32)

    in_maps = []
    for c in range(NCORES):
        b, tp = c // 2, c % 2
        hs = tp * 512
        # xt: [D, S] -> [et, 128, S] -> [128, et*S]
        xt = np.ascontiguousarray(
            x[b].T.reshape(ET, P, S).transpose(1, 0, 2)).astype(bf)
        # wq/wk: W[hs:hs+512, :].T = [e, hd] -> [et, 128, pair, 128] -> p-first
        wq = np.ascontiguousarray(
            Wq[hs:hs + 512, :].T.reshape(ET, P, NPAIR, P)
            .transpose(1, 0, 2, 3)).astype(bf)
        wk = np.ascontiguousarray(
            Wk[hs:hs + 512, :].T.reshape(ET, P, NPAIR, P)
            .transpose(1, 0, 2, 3)).astype(bf)
        wv = np.ascontiguousarray(
            Wv[hs:hs + 512, :].T.reshape(ET, P, 512)
            .transpose(1, 0, 2)).astype(bf)
        # wo: Wo[:, hs:hs+512].T = [hd, e] -> [hdtile, 128, 1024] -> p-first
        wo = np.ascontiguousarray(
            Wo[:, hs:hs + 512].T.reshape(NPAIR, P, D)
            .transpose(1, 0, 2)).astype(bf)
        in_maps.append({
            "xt": xt.reshape(P, ET * S),
            "wq": wq.reshape(P, ET * NPAIR * P),
            "wk": wk.reshape(P, ET * NPAIR * P),
            "wv": wv.reshape(P, ET * 512),
            "wo": wo.reshape(P, NPAIR * D),
        })

    trace = bool(os.environ.get("BASS_TRACE"))
    res = run_bass_kernel_spmd(nc, in_maps, core_ids=list(range(NCORES)),
                               trace=trace)
    LAST_EXEC_TIME_NS = res.exec_time_ns

    outs = [np.asarray(r["out"], np.float32) for r in res.results]
    return np.stack([outs[2 * b] + outs[2 * b + 1] for b in range(B)])


# revision 4
# speedup vs baseline: 1.4255x; 1.0305x over previous
"""Multi-head attention (B=4, S=2048, D=1024, H=16) on 8 trn2 NeuronCores.

Sharding (v2): data-parallel over batch x 2-way tensor-parallel over heads.
Core c owns batch b = c//2 and heads [8*(c%2), 8*(c%2)+8) (= model dims
[512*(c%2), 512*(c%2)+512)).  Each core: q/k/v projections for its 8 heads
over its batch's 2048 tokens, attention, partial out-projection against its
512 columns of Wo.  Host sums the 2 partial outputs per batch (all-reduce
of the Megatron pattern at gather time).  vs 8-way head-TP this cuts
per-core HBM traffic 4x (4.2MB bf16 in + 8.4MB f32 out).

Per-core kernel:
  - Heads processed in 4 PAIRS.  Score matmuls have contraction = HD = 64,
    so each kt's two heads run as ROW-TILED CONCURRENT matmuls (PE row
    groups 0-63 / 64-127, auto-derived from base partitions) into separate
    PSUM tiles -> ~2x score throughput vs serial 64-contraction matmuls.
  - exp on ScalarE per (kt, head) [128,1024] tile; ACTIVATEs pipeline at
    ~(N+171)/1.2ns, so this costs only ~7% over 2048-wide tiles while
    halving PSUM (2 banks/tile).
  - v projection in FLIPPED layout (lhsT = x chunk, rhs = Wv): v lands
    [tokens, dims] directly -- zero PE transposes.  A ones column per head
    in v_ext makes attn@v emit numerator + softmax denominator together.
  - attn@v per (head, qc-half, kt-half): 8-matmul groups into one rotating
    PSUM bank, DVE-evicted/accumulated into SBUF f32.  The kt-split lets
    the first half run inside its own stretch, halving exp liveness.
  - division: DVE fast-reciprocal of the denominator row, GPSIMD
    partition_broadcast to 64 rows, DVE multiply into outT (bf16).
  - out-proj contracts all 512 head dims (all pairs) -> runs late; f32
    [2048, 1024] partials out, host adds core pairs.

PSUM (8 banks): scores 3x[128,1024] (6) + attnv 1x[128,512] + shared
proj/oproj 1x[128,512].  An emission-order scheduler pumps an urgent queue
(attnv/divisions -- free exp tiles + PSUM) and a background queue
(projections, out-proj) between score/exp emissions to keep the PE dense
(HAM stays warm) while ScalarE paces the pipeline.
"""
import os
import sys

sys.path.insert(0, "/opt/trn_rl_repo")

from collections import deque
from contextlib import ExitStack

import numpy as np
import ml_dtypes

import concourse.mybir as mybir
import concourse.tile as tile
from concourse import bacc
from concourse._compat import with_exitstack
from concourse.bass_utils import run_bass_kernel_spmd

B, S, D, H = 4, 2048, 1024, 16
HD = D // H              # 64
P = 128
NCORES = 8
ET = D // P              # 8 contraction e-tiles
NPAIR = 4                # head pairs per core (8 heads)
KT = S // P              # 16 key tiles
QC = 1024                # query chunk (stretch granularity)
NQC = S // QC            # 2
TC = 512                 # q/k projection token chunk
VW = 8 * (HD + 1)        # v_ext cols per kt = 520
EXP_SCALE = float(1.0 / np.sqrt(HD))
EPOOL_BUFS = 27

f32 = mybir.dt.float32
bf16 = mybir.dt.bfloat16
Exp = mybir.ActivationFunctionType.Exp

LAST_EXEC_TIME_NS = None
_CACHED_NC = None


@with_exitstack
def _mha_kernel(ctx: ExitStack, tc_: tile.TileContext, ins, outs):
    nc = tc_.nc
    xt_d, wq_d, wk_d, wv_d, wo_d = ins
    out_d = outs[0]

    xpool = ctx.enter_context(tc_.tile_pool(name="xpool", bufs=1))
    wpool = ctx.enter_context(tc_.tile_pool(name="wpool", bufs=1))
    qkpool = ctx.enter_context(tc_.tile_pool(name="qkpool", bufs=2))
    vxpool = ctx.enter_context(tc_.tile_pool(name="vxpool", bufs=1))
    opool = ctx.enter_context(tc_.tile_pool(name="opool", bufs=1))
    ocpool = ctx.enter_context(tc_.tile_pool(name="ocpool", bufs=2))
    dpool = ctx.enter_context(tc_.tile_pool(name="dpool", bufs=2))
    rbpool = ctx.enter_context(tc_.tile_pool(name="rbpool", bufs=2))
    ospool = ctx.enter_context(tc_.tile_pool(name="ospool", bufs=4))
    epool = ctx.enter_context(tc_.tile_pool(name="epool", bufs=EPOOL_BUFS))

    scp = ctx.enter_context(tc_.tile_pool(name="scp", bufs=3, space="PSUM"))
    oep = ctx.enter_context(tc_.tile_pool(name="oep", bufs=1, space="PSUM"))
    mpp = ctx.enter_context(tc_.tile_pool(name="mpp", bufs=1, space="PSUM"))

    # flat 2D layouts; host pre-arranges to match
    xt = xpool.tile([P, ET * S], bf16, tag="xt")            # [e-tile, tok]
    wq = wpool.tile([P, ET * NPAIR * P], bf16, tag="wq")    # [et, pair, hd]
    wk = wpool.tile([P, ET * NPAIR * P], bf16, tag="wk")
    wv = wpool.tile([P, ET * 512], bf16, tag="wvo")         # slot reused by wo
    v_ext = vxpool.tile([P, KT * VW], bf16, tag="vx")       # [kt, 8h, 65]
    outT = opool.tile([P, NPAIR * S], bf16, tag="outT")     # [hd-tile, tok]

    nc.gpsimd.dma_start(wk[:], wk_d[:])
    # x streamed in 4 token-chunks so PK(c) can start as soon as chunk c
    # lands: xt layout [128, et, tok] -> chunk c covers tok [c*512,(c+1)*512)
    xt3 = xt[:].rearrange("p (e t) -> p e t", e=ET)
    xd3 = xt_d[:].rearrange("p (e t) -> p e t", e=ET)
    for c in range(4):
        nc.gpsimd.dma_start(xt3[:, :, c * TC:(c + 1) * TC],
                            xd3[:, :, c * TC:(c + 1) * TC])
    nc.sync.dma_start(wq[:], wq_d[:])
    nc.sync.dma_start(wv[:], wv_d[:])

    # ones columns of v_ext (col 64 of each head block), set once
    vcols = v_ext[:].rearrange("p (kh c) -> p kh c", c=HD + 1)
    nc.vector.memset(vcols[:, :, HD:HD + 1], 1.0)

    qTs, kTs = {}, {}
    exps = {}
    oecps, denss = {}, {}
    wo_box = {}

    # ---------------- unit bodies ----------------
    def alloc_qk(p):
        if p not in kTs:
            qTs[p] = qkpool.tile([P, S], bf16, tag="qT", name=f"qT{p}")
            kTs[p] = qkpool.tile([P, S], bf16, tag="kT", name=f"kT{p}")

    def proj_qk(w, dst, p, c):
        # one 8-matmul accumulation group: [128 pair-dims, 512 tokens]
        pp = mpp.tile([P, TC], f32, tag="mp")
        for et in range(ET):
            nc.tensor.matmul(
                pp[:],
                w[:, (et * NPAIR + p) * P:(et * NPAIR + p + 1) * P],
                xt[:, et * S + c * TC: et * S + (c + 1) * TC],
                start=(et == 0), stop=(et == ET - 1),
            )
        nc.vector.tensor_copy(dst[:, c * TC:(c + 1) * TC], pp[:])

    def proj_v(c):
        # flipped: [128 tokens of kt-tile c, 512 v-dims]
        pv = mpp.tile([P, TC], f32, tag="mp")
        for et in range(ET):
            nc.tensor.matmul(
                pv[:],
                xt[:, et * S + c * P: et * S + (c + 1) * P],
                wv[:, et * 512:(et + 1) * 512],
                start=(et == 0), stop=(et == ET - 1),
            )
        dst = v_ext[:, c * VW:(c + 1) * VW].rearrange(
            "p (h c2) -> p h c2", c2=HD + 1)[:, :, 0:HD]
        nc.vector.tensor_copy(dst, pv[:].rearrange("p (h c2) -> p h c2", c2=HD))

    def load_wo():
        wo = wpool.tile([P, NPAIR * D], bf16, tag="wvo", name="wo")
        nc.sync.dma_start(wo[:], wo_d[:])
        wo_box["wo"] = wo

    def scores_unit(p, qc, kt):
        # row-tiled concurrent head pair: h0 rows 0-63, h1 rows 64-127
        sc0 = scp.tile([P, QC], f32, tag="sc", name=f"sc{p}_{qc}_{kt}_0")
        sc1 = scp.tile([P, QC], f32, tag="sc", name=f"sc{p}_{qc}_{kt}_1")
        kTp, qTp = kTs[p], qTs[p]
        for l in range(2):
            for h, sc in ((0, sc0), (1, sc1)):
                rows = slice(h * HD, (h + 1) * HD)
                nc.tensor.matmul(
                    sc[:, l * 512:(l + 1) * 512],
                    kTp[rows, kt * P:(kt + 1) * P],
                    qTp[rows, qc * QC + l * 512: qc * QC + (l + 1) * 512],
                    start=True, stop=True,
                )
        for h, sc in ((0, sc0), (1, sc1)):
            ex = epool.tile([P, QC], bf16, tag="exp", name=f"ex{p}_{qc}_{kt}_{h}")
            nc.scalar.activation(ex[:], sc[:], Exp, scale=EXP_SCALE)
            exps[(p, qc, kt, h)] = ex

    def attnv_unit(p, qc, h, l, kh):
        # one contiguous 8-matmul accumulation group over kt half kh
        key = (p, qc, h)
        if key not in oecps:
            oecps[key] = ocpool.tile([P, QC], f32, tag="ocp",
                                     name=f"ocp{p}_{qc}_{h}")
            denss[key] = dpool.tile([1, QC], f32, tag="dens",
                                    name=f"den{p}_{qc}_{h}")
        oe = oep.tile([P, 512], f32, tag="oe")
        base = (2 * p + h) * (HD + 1)
        for i in range(8):
            kt = kh * 8 + i
            nc.tensor.matmul(
                oe[0:HD + 1, :],
                v_ext[:, kt * VW + base: kt * VW + base + HD + 1],
                exps[(p, qc, kt, h)][:, l * 512:(l + 1) * 512],
                start=(i == 0), stop=(i == 7),
            )
        ocp, dn = oecps[key], denss[key]
        ls = slice(l * 512, (l + 1) * 512)
        if kh == 0:
            nc.vector.tensor_copy(ocp[0:HD, ls], oe[0:HD, :])
            nc.vector.tensor_copy(dn[0:1, ls], oe[HD:HD + 1, :])
        else:
            nc.vector.tensor_add(ocp[0:HD, ls], ocp[0:HD, ls], oe[0:HD, :])
            nc.vector.tensor_add(dn[0:1, ls], dn[0:1, ls], oe[HD:HD + 1, :])

    def div_unit(p, qc, h):
        key = (p, qc, h)
        rec = dpool.tile([1, QC], f32, tag="recs", name=f"rec{p}_{qc}_{h}")
        nc.vector.reciprocal_approx_fast(rec[:], denss[key][:])
        rb = rbpool.tile([HD, QC], f32, tag="rb", name=f"rb{p}_{qc}_{h}")
        nc.gpsimd.partition_broadcast(rb[:], rec[:])
        dst = outT[h * HD:(h + 1) * HD, p * S + qc * QC: p * S + (qc + 1) * QC]
        nc.vector.tensor_mul(dst, oecps[key][0:HD, :], rb[:])

    in_drain = [False]

    def oproj_unit(qc, tc):
        t0 = (qc * 8 + tc) * P
        wo = wo_box["wo"]
        osb = ospool.tile([P, D], f32, tag="osb")
        for eh in range(2):
            po = (scp.tile([P, QC], f32, tag="sc", name=f"po{qc}_{tc}_{eh}")
                  if in_drain[0] else mpp.tile([P, TC], f32, tag="mp"))
            for ht in range(NPAIR):
                nc.tensor.matmul(
                    po[0:P, 0:TC],
                    outT[:, ht * S + t0: ht * S + t0 + P],
                    wo[:, ht * D + eh * 512: ht * D + (eh + 1) * 512],
                    start=(ht == 0), stop=(ht == NPAIR - 1),
                )
            nc.vector.tensor_copy(osb[:, eh * 512:(eh + 1) * 512],
                                  po[0:P, 0:TC])
        eng = nc.sync if tc % 2 == 0 else nc.gpsimd
        eng.dma_start(out_d[t0:t0 + P, :], osb[:])

    # ---------------- scheduler ----------------
    urgent = deque()   # (fn, cycles)
    backg = deque()    # (fn, cycles, ready_si, tag)
    cur_si = 0
    budget = 0.0

    def pump(room):
        nonlocal budget
        budget = min(budget + room, 6000.0)
        while budget > 0:
            if urgent:
                fn, cyc = urgent.popleft()
            elif backg and backg[0][2] <= cur_si:
                fn, cyc, _, _ = backg.popleft()
            else:
                break
            fn()
            budget -= cyc

    def pump_until(tag):
        # emit queued units in order until no `tag` units remain in backg
        while any(t == tag for _, _, _, t in backg):
            if urgent:
                fn, cyc = urgent.popleft()
            else:
                fn, cyc, _, _ = backg.popleft()
            fn()

    # ---------------- lead-in ----------------
    alloc_qk(0)
    proj_qk(wk, kTs[0], 0, 0)
    proj_qk(wq, qTs[0], 0, 0)
    proj_qk(wq, qTs[0], 0, 1)
    for c in range(1, 4):
        backg.append((
            (lambda cc: lambda: proj_qk(wk, kTs[0], 0, cc))(c), 4400, 0, "kq0"))

    backg.append((lambda: proj_qk(wq, qTs[0], 0, 2), 4400, 0, "kq0"))
    backg.append((lambda: proj_qk(wq, qTs[0], 0, 3), 4400, 0, "kq0"))
    for c in range(KT):
        backg.append(((lambda cc: lambda: proj_v(cc))(c), 4400, 0,
                      "pv0" if c < 8 else "pv1"))
    backg.append((load_wo, 100, 0, "wo"))
    for p in range(1, NPAIR):
        ready = max(0, 2 * p - 2)
        for c in range(4):
            backg.append((
                (lambda pp, cc: lambda: (alloc_qk(pp),
                                         proj_qk(wk, kTs[pp], pp, cc))[-1])(p, c),
                4400, ready, f"kq{p}"))
        for c in range(4):
            backg.append((
                (lambda pp, cc: lambda: proj_qk(wq, qTs[pp], pp, cc))(p, c),
                4400, ready, f"kq{p}"))

    # ---------------- stretches ----------------
    stretches = [(p, qc) for p in range(NPAIR) for qc in range(NQC)]
    for si, (p, qc) in enumerate(stretches):
        cur_si = si
        if qc == 0 and p > 0:
            pump_until(f"kq{p}")   # scores(p) need qT/kT(p) emitted first
        for kt in range(KT):
            scores_unit(p, qc, kt)
            if kt == 7 and si == 0:
                pump_until("pv0")  # attnv kt 0-7 needs v_ext chunks 0-7
            if kt in (7, 9, 11, 13):
                h, l = divmod((kt - 7) // 2, 2)
                urgent.append((
                    (lambda a, b, c2, d: lambda: attnv_unit(a, b, c2, d, 0)
                     )(p, qc, h, l), 4400))
            pump(4600)
        # second kt-halves + divisions, consumed during the next stretch
        if si == 0:
            pump_until("pv1")  # attnv kt 8-15 needs v_ext chunks 8-15
        for h in range(2):
            for l in range(2):
                urgent.append((
                    (lambda a, b, c2, d: lambda: attnv_unit(a, b, c2, d, 1)
                     )(p, qc, h, l), 4400))
            urgent.append((
                (lambda a, b, c2: lambda: div_unit(a, b, c2))(p, qc, h), 600))
        if p == NPAIR - 1:
            # out-proj for this qc: available once p3's divisions (just
            # queued ahead of these in-order) have been emitted
            for tc in range(8):
                backg.append((
                    (lambda q2, t2: lambda: oproj_unit(q2, t2))(qc, tc),
                    4800, si, "po"))

    cur_si = len(stretches)
    in_drain[0] = True
    while urgent or backg:
        pump(10000)


def _build():
    global _CACHED_NC
    if _CACHED_NC is not None:
        return _CACHED_NC
    nc = bacc.Bacc("TRN2", target_bir_lowering=False, debug=False)
    xt = nc.dram_tensor("xt", [P, ET * S], bf16, kind="ExternalInput").ap()
    wq = nc.dram_tensor("wq", [P, ET * NPAIR * P], bf16,
                        kind="ExternalInput").ap()
    wk = nc.dram_tensor("wk", [P, ET * NPAIR * P], bf16,
                        kind="ExternalInput").ap()
    wv = nc.dram_tensor("wv", [P, ET * 512], bf16, kind="ExternalInput").ap()
    wo = nc.dram_tensor("wo", [P, NPAIR * D], bf16, kind="ExternalInput").ap()
    out = nc.dram_tensor("out", [S, D], f32, kind="ExternalOutput").ap()

    with tile.TileContext(nc) as tc_:
        _mha_kernel(tc_, [xt, wq, wk, wv, wo], [out])
    nc.compile()
    _CACHED_NC = nc
    return nc


def kernel(x: np.ndarray, Wq: np.ndarray, Wk: np.ndarray, Wv: np.ndarray,
           Wo: np.ndarray) -> np.ndarray:
    global LAST_EXEC_TIME_NS
    nc = _build()
    bf = ml_dtypes.bfloat16

    x = np.asarray(x, dtype=np.float32)
    Wq = np.asarray(Wq, np.float32)
    Wk = np.asarray(Wk, np.float32)
    Wv = np.asarray(Wv, np.float32)
    Wo = np.asarray(Wo, np.float32)

    in_maps = []
    for c in range(NCORES):
        b, tp = c // 2, c % 2
        hs = tp * 512
        # xt: [D, S] -> [et, 128, S] -> [128, et*S]
        xt = np.ascontiguousarray(
            x[b].T.reshape(ET, P, S).transpose(1, 0, 2)).astype(bf)
        # wq/wk: W[hs:hs+512, :].T = [e, hd] -> [et, 128, pair, 128] -> p-first
        wq = np.ascontiguousarray(
            Wq[hs:hs + 512, :].T.reshape(ET, P, NPAIR, P)
            .transpose(1, 0, 2, 3)).astype(bf)
        wk = np.ascontiguousarray(
            Wk[hs:hs + 512, :].T.reshape(ET, P, NPAIR, P)
            .transpose(1, 0, 2, 3)).astype(bf)
        wv = np.ascontiguousarray(
            Wv[hs:hs + 512, :].T.reshape(ET, P, 512)
            .transpose(1, 0, 2)).astype(bf)
        # wo: Wo[:, hs:hs+512].T = [hd, e] -> [hdtile, 128, 1024] -> p-first
        wo = np.ascontiguousarray(
            Wo[:, hs:hs + 512].T.reshape(NPAIR, P, D)
            .transpose(1, 0, 2)).astype(bf)
        in_maps.append({
            "xt": xt.reshape(P, ET * S),
            "wq": wq.reshape(P, ET * NPAIR * P),
            "wk": wk.reshape(P, ET * NPAIR * P),
            "wv": wv.reshape(P, ET * 512),
            "wo": wo.reshape(P, NPAIR * D),
        })

    trace = bool(os.environ.get("BASS_TRACE"))
    res = run_bass_kernel_spmd(nc, in_maps, core_ids=list(range(NCORES)),
                               trace=trace)
    LAST_EXEC_TIME_NS = res.exec_time_ns

    outs = [np.asarray(r["out"], np.float32) for r in res.results]
    return np.stack([outs[2 * b] + outs[2 * b + 1] for b in range(B)])


# revision 5
# speedup vs baseline: 1.4363x; 1.0076x over previous
"""Multi-head attention (B=4, S=2048, D=1024, H=16) on 8 trn2 NeuronCores.

Sharding (v2): data-parallel over batch x 2-way tensor-parallel over heads.
Core c owns batch b = c//2 and heads [8*(c%2), 8*(c%2)+8) (= model dims
[512*(c%2), 512*(c%2)+512)).  Each core: q/k/v projections for its 8 heads
over its batch's 2048 tokens, attention, partial out-projection against its
512 columns of Wo.  Host sums the 2 partial outputs per batch (all-reduce
of the Megatron pattern at gather time).  vs 8-way head-TP this cuts
per-core HBM traffic 4x (4.2MB bf16 in + 8.4MB f32 out).

Per-core kernel:
  - Heads processed in 4 PAIRS.  Score matmuls have contraction = HD = 64,
    so each kt's two heads run as ROW-TILED CONCURRENT matmuls (PE row
    groups 0-63 / 64-127, auto-derived from base partitions) into separate
    PSUM tiles -> ~2x score throughput vs serial 64-contraction matmuls.
  - exp on ScalarE per (kt, head) [128,1024] tile; ACTIVATEs pipeline at
    ~(N+171)/1.2ns, so this costs only ~7% over 2048-wide tiles while
    halving PSUM (2 banks/tile).
  - v projection in FLIPPED layout (lhsT = x chunk, rhs = Wv): v lands
    [tokens, dims] directly -- zero PE transposes.  A ones column per head
    in v_ext makes attn@v emit numerator + softmax denominator together.
  - attn@v per (head, qc-half, kt-half): 8-matmul groups into one rotating
    PSUM bank, DVE-evicted/accumulated into SBUF f32.  The kt-split lets
    the first half run inside its own stretch, halving exp liveness.
  - division: DVE fast-reciprocal of the denominator row, GPSIMD
    partition_broadcast to 64 rows, DVE multiply into outT (bf16).
  - out-proj contracts all 512 head dims (all pairs) -> runs late; f32
    [2048, 1024] partials out, host adds core pairs.

PSUM (8 banks): scores 3x[128,1024] (6) + attnv 1x[128,512] + shared
proj/oproj 1x[128,512].  An emission-order scheduler pumps an urgent queue
(attnv/divisions -- free exp tiles + PSUM) and a background queue
(projections, out-proj) between score/exp emissions to keep the PE dense
(HAM stays warm) while ScalarE paces the pipeline.
"""
import os
import sys

sys.path.insert(0, "/opt/trn_rl_repo")

from collections import deque
from contextlib import ExitStack

import numpy as np
import ml_dtypes

import concourse.mybir as mybir
import concourse.tile as tile
from concourse import bacc
from concourse._compat import with_exitstack
from concourse.bass_utils import run_bass_kernel_spmd

B, S, D, H = 4, 2048, 1024, 16
HD = D // H              # 64
P = 128
NCORES = 8
ET = D // P              # 8 contraction e-tiles
NPAIR = 4                # head pairs per core (8 heads)
KT = S // P              # 16 key tiles
QC = 1024                # query chunk (stretch granularity)
NQC = S // QC            # 2
TC = 512                 # q/k projection token chunk
VW = 8 * (HD + 1)        # v_ext cols per kt = 520
EXP_SCALE = float(1.0 / np.sqrt(HD))
EPOOL_BUFS = 27

f32 = mybir.dt.float32
bf16 = mybir.dt.bfloat16
Exp = mybir.ActivationFunctionType.Exp

LAST_EXEC_TIME_NS = None
_CACHED_NC = None


@with_exitstack
def _mha_kernel(ctx: ExitStack, tc_: tile.TileContext, ins, outs):
    nc = tc_.nc
    xt_d, wq_d, wk_d, wv_d, wo_d = ins
    out_d = outs[0]

    xpool = ctx.enter_context(tc_.tile_pool(name="xpool", bufs=1))
    wpool = ctx.enter_context(tc_.tile_pool(name="wpool", bufs=1))
    qkpool = ctx.enter_context(tc_.tile_pool(name="qkpool", bufs=2))
    vxpool = ctx.enter_context(tc_.tile_pool(name="vxpool", bufs=1))
    opool = ctx.enter_context(tc_.tile_pool(name="opool", bufs=1))
    ocpool = ctx.enter_context(tc_.tile_pool(name="ocpool", bufs=2))
    dpool = ctx.enter_context(tc_.tile_pool(name="dpool", bufs=2))
    rbpool = ctx.enter_context(tc_.tile_pool(name="rbpool", bufs=2))
    ospool = ctx.enter_context(tc_.tile_pool(name="ospool", bufs=4))
    epool = ctx.enter_context(tc_.tile_pool(name="epool", bufs=EPOOL_BUFS))

    scp = ctx.enter_context(tc_.tile_pool(name="scp", bufs=3, space="PSUM"))
    oep = ctx.enter_context(tc_.tile_pool(name="oep", bufs=1, space="PSUM"))
    mpp = ctx.enter_context(tc_.tile_pool(name="mpp", bufs=1, space="PSUM"))

    # flat 2D layouts; host pre-arranges to match
    xt = xpool.tile([P, ET * S], bf16, tag="xt")            # [e-tile, tok]
    wq = wpool.tile([P, ET * NPAIR * P], bf16, tag="wq")    # [et, pair, hd]
    wk = wpool.tile([P, ET * NPAIR * P], bf16, tag="wk")
    wv = wpool.tile([P, ET * 512], bf16, tag="wvo")         # slot reused by wo
    v_ext = vxpool.tile([P, KT * VW], bf16, tag="vx")       # [kt, 8h, 65]
    outT = opool.tile([P, NPAIR * S], bf16, tag="outT")     # [hd-tile, tok]

    # pair-striped weight loads + half-chunked first x load: the first
    # PK/PQ groups need only pair-0 weight columns and tokens 0-255
    wk4 = wk[:].rearrange("p (e pr c) -> p e pr c", pr=NPAIR, c=P)
    wkd4 = wk_d[:].rearrange("p (e pr c) -> p e pr c", pr=NPAIR, c=P)
    wq4 = wq[:].rearrange("p (e pr c) -> p e pr c", pr=NPAIR, c=P)
    wqd4 = wq_d[:].rearrange("p (e pr c) -> p e pr c", pr=NPAIR, c=P)
    xt3 = xt[:].rearrange("p (e t) -> p e t", e=ET)
    xd3 = xt_d[:].rearrange("p (e t) -> p e t", e=ET)
    nc.gpsimd.dma_start(wk4[:, :, 0:1, :], wkd4[:, :, 0:1, :])
    nc.gpsimd.dma_start(xt3[:, :, 0:256], xd3[:, :, 0:256])
    nc.gpsimd.dma_start(xt3[:, :, 256:512], xd3[:, :, 256:512])
    nc.gpsimd.dma_start(xt3[:, :, 512:1024], xd3[:, :, 512:1024])
    nc.gpsimd.dma_start(wk4[:, :, 1:4, :], wkd4[:, :, 1:4, :])
    for c in range(2, 4):
        nc.gpsimd.dma_start(xt3[:, :, c * TC:(c + 1) * TC],
                            xd3[:, :, c * TC:(c + 1) * TC])
    nc.sync.dma_start(wq4[:, :, 0:1, :], wqd4[:, :, 0:1, :])
    nc.sync.dma_start(wq4[:, :, 1:4, :], wqd4[:, :, 1:4, :])
    nc.sync.dma_start(wv[:], wv_d[:])

    # ones columns of v_ext (col 64 of each head block), set once
    vcols = v_ext[:].rearrange("p (kh c) -> p kh c", c=HD + 1)
    nc.vector.memset(vcols[:, :, HD:HD + 1], 1.0)

    qTs, kTs = {}, {}
    exps = {}
    oecps, denss = {}, {}
    wo_box = {}

    # ---------------- unit bodies ----------------
    def alloc_qk(p):
        if p not in kTs:
            qTs[p] = qkpool.tile([P, S], bf16, tag="qT", name=f"qT{p}")
            kTs[p] = qkpool.tile([P, S], bf16, tag="kT", name=f"kT{p}")

    def proj_qk(w, dst, p, c, t0=0, tw=TC):
        # one 8-matmul accumulation group: [128 pair-dims, tw tokens]
        pp = mpp.tile([P, TC], f32, tag="mp")
        base = c * TC + t0
        for et in range(ET):
            nc.tensor.matmul(
                pp[0:P, 0:tw],
                w[:, (et * NPAIR + p) * P:(et * NPAIR + p + 1) * P],
                xt[:, et * S + base: et * S + base + tw],
                start=(et == 0), stop=(et == ET - 1),
            )
        nc.vector.tensor_copy(dst[:, base:base + tw], pp[0:P, 0:tw])

    def proj_v(c):
        # flipped: [128 tokens of kt-tile c, 512 v-dims]
        pv = mpp.tile([P, TC], f32, tag="mp")
        for et in range(ET):
            nc.tensor.matmul(
                pv[:],
                xt[:, et * S + c * P: et * S + (c + 1) * P],
                wv[:, et * 512:(et + 1) * 512],
                start=(et == 0), stop=(et == ET - 1),
            )
        dst = v_ext[:, c * VW:(c + 1) * VW].rearrange(
            "p (h c2) -> p h c2", c2=HD + 1)[:, :, 0:HD]
        nc.vector.tensor_copy(dst, pv[:].rearrange("p (h c2) -> p h c2", c2=HD))

    def load_wo():
        wo = wpool.tile([P, NPAIR * D], bf16, tag="wvo", name="wo")
        nc.sync.dma_start(wo[:], wo_d[:])
        wo_box["wo"] = wo

    def scores_unit(p, qc, kt):
        # row-tiled concurrent head pair: h0 rows 0-63, h1 rows 64-127
        sc0 = scp.tile([P, QC], f32, tag="sc", name=f"sc{p}_{qc}_{kt}_0")
        sc1 = scp.tile([P, QC], f32, tag="sc", name=f"sc{p}_{qc}_{kt}_1")
        kTp, qTp = kTs[p], qTs[p]
        for l in range(2):
            for h, sc in ((0, sc0), (1, sc1)):
                rows = slice(h * HD, (h + 1) * HD)
                nc.tensor.matmul(
                    sc[:, l * 512:(l + 1) * 512],
                    kTp[rows, kt * P:(kt + 1) * P],
                    qTp[rows, qc * QC + l * 512: qc * QC + (l + 1) * 512],
                    start=True, stop=True,
                )
        for h, sc in ((0, sc0), (1, sc1)):
            ex = epool.tile([P, QC], bf16, tag="exp", name=f"ex{p}_{qc}_{kt}_{h}")
            nc.scalar.activation(ex[:], sc[:], Exp, scale=EXP_SCALE)
            exps[(p, qc, kt, h)] = ex

    def attnv_unit(p, qc, h, l, kh):
        # one contiguous 8-matmul accumulation group over kt half kh
        key = (p, qc, h)
        if key not in oecps:
            oecps[key] = ocpool.tile([P, QC], f32, tag="ocp",
                                     name=f"ocp{p}_{qc}_{h}")
            denss[key] = dpool.tile([1, QC], f32, tag="dens",
                                    name=f"den{p}_{qc}_{h}")
        oe = oep.tile([P, 512], f32, tag="oe")
        base = (2 * p + h) * (HD + 1)
        for i in range(8):
            kt = kh * 8 + i
            nc.tensor.matmul(
                oe[0:HD + 1, :],
                v_ext[:, kt * VW + base: kt * VW + base + HD + 1],
                exps[(p, qc, kt, h)][:, l * 512:(l + 1) * 512],
                start=(i == 0), stop=(i == 7),
            )
        ocp, dn = oecps[key], denss[key]
        ls = slice(l * 512, (l + 1) * 512)
        if kh == 0:
            nc.vector.tensor_copy(ocp[0:HD, ls], oe[0:HD, :])
            nc.vector.tensor_copy(dn[0:1, ls], oe[HD:HD + 1, :])
        else:
            nc.vector.tensor_add(ocp[0:HD, ls], ocp[0:HD, ls], oe[0:HD, :])
            nc.vector.tensor_add(dn[0:1, ls], dn[0:1, ls], oe[HD:HD + 1, :])

    def div_unit(p, qc, h):
        key = (p, qc, h)
        rec = dpool.tile([1, QC], f32, tag="recs", name=f"rec{p}_{qc}_{h}")
        nc.vector.reciprocal_approx_fast(rec[:], denss[key][:])
        rb = rbpool.tile([HD, QC], f32, tag="rb", name=f"rb{p}_{qc}_{h}")
        nc.gpsimd.partition_broadcast(rb[:], rec[:])
        dst = outT[h * HD:(h + 1) * HD, p * S + qc * QC: p * S + (qc + 1) * QC]
        nc.vector.tensor_mul(dst, oecps[key][0:HD, :], rb[:])

    in_drain = [False]

    def oproj_unit(qc, tc):
        t0 = (qc * 8 + tc) * P
        wo = wo_box["wo"]
        osb = ospool.tile([P, D], f32, tag="osb")
        for eh in range(2):
            po = (scp.tile([P, QC], f32, tag="sc", name=f"po{qc}_{tc}_{eh}")
                  if in_drain[0] else mpp.tile([P, TC], f32, tag="mp"))
            for ht in range(NPAIR):
                nc.tensor.matmul(
                    po[0:P, 0:TC],
                    outT[:, ht * S + t0: ht * S + t0 + P],
                    wo[:, ht * D + eh * 512: ht * D + (eh + 1) * 512],
                    start=(ht == 0), stop=(ht == NPAIR - 1),
                )
            nc.vector.tensor_copy(osb[:, eh * 512:(eh + 1) * 512],
                                  po[0:P, 0:TC])
        eng = nc.sync if tc % 2 == 0 else nc.gpsimd
        eng.dma_start(out_d[t0:t0 + P, :], osb[:])

    # ---------------- scheduler ----------------
    urgent = deque()   # (fn, cycles)
    backg = deque()    # (fn, cycles, ready_si, tag)
    cur_si = 0
    budget = 0.0

    def pump(room):
        nonlocal budget
        budget = min(budget + room, 6000.0)
        while budget > 0:
            if urgent:
                fn, cyc = urgent.popleft()
            elif backg and backg[0][2] <= cur_si:
                fn, cyc, _, _ = backg.popleft()
            else:
                break
            fn()
            budget -= cyc

    def pump_until(tag):
        # emit queued units in order until no `tag` units remain in backg
        while any(t == tag for _, _, _, t in backg):
            if urgent:
                fn, cyc = urgent.popleft()
            else:
                fn, cyc, _, _ = backg.popleft()
            fn()

    # ---------------- lead-in ----------------
    alloc_qk(0)
    proj_qk(wk, kTs[0], 0, 0, 0, 256)
    proj_qk(wq, qTs[0], 0, 0, 0, 256)
    proj_qk(wk, kTs[0], 0, 0, 256, 256)
    proj_qk(wq, qTs[0], 0, 0, 256, 256)
    proj_qk(wq, qTs[0], 0, 1)

    pv_unit = lambda cc: ((lambda: proj_v(cc)), 4400, 0,
                          "pv0" if cc < 8 else "pv1")
    backg.append((lambda: proj_qk(wk, kTs[0], 0, 1), 4400, 0, "kq0"))
    for c in range(0, 4):
        backg.append(pv_unit(c))
    backg.append((lambda: proj_qk(wk, kTs[0], 0, 2), 4400, 0, "kq0"))
    for c in range(4, 8):
        backg.append(pv_unit(c))
    backg.append((lambda: proj_qk(wk, kTs[0], 0, 3), 4400, 0, "kq0"))
    backg.append((lambda: proj_qk(wq, qTs[0], 0, 2), 4400, 0, "kq0"))
    backg.append((lambda: proj_qk(wq, qTs[0], 0, 3), 4400, 0, "kq0"))
    for c in range(8, KT):
        backg.append(pv_unit(c))
    backg.append((load_wo, 100, 0, "wo"))
    for p in range(1, NPAIR):
        ready = 2 * p - 1
        for c in range(4):
            backg.append((
                (lambda pp, cc: lambda: (alloc_qk(pp),
                                         proj_qk(wk, kTs[pp], pp, cc))[-1])(p, c),
                4400, ready, f"kq{p}"))
        for c in range(4):
            backg.append((
                (lambda pp, cc: lambda: proj_qk(wq, qTs[pp], pp, cc))(p, c),
                4400, ready, f"kq{p}"))

    # ---------------- stretches ----------------
    stretches = [(p, qc) for p in range(NPAIR) for qc in range(NQC)]
    for si, (p, qc) in enumerate(stretches):
        cur_si = si
        if qc == 0 and p > 0:
            pump_until(f"kq{p}")   # scores(p) need qT/kT(p) emitted first
        for kt in range(KT):
            scores_unit(p, qc, kt)
            if kt == 7 and si == 0:
                pump_until("pv0")  # attnv kt 0-7 needs v_ext chunks 0-7
            if kt in (7, 9, 11, 13):
                h, l = divmod((kt - 7) // 2, 2)
                urgent.append((
                    (lambda a, b, c2, d: lambda: attnv_unit(a, b, c2, d, 0)
                     )(p, qc, h, l), 4400))
            pump(4600)
        # second kt-halves + divisions, consumed during the next stretch
        if si == 0:
            pump_until("pv1")  # attnv kt 8-15 needs v_ext chunks 8-15
        for h in range(2):
            for l in range(2):
                urgent.append((
                    (lambda a, b, c2, d: lambda: attnv_unit(a, b, c2, d, 1)
                     )(p, qc, h, l), 4400))
            urgent.append((
                (lambda a, b, c2: lambda: div_unit(a, b, c2))(p, qc, h), 600))
        if p == NPAIR - 1:
            # out-proj for this qc: available once p3's divisions (just
            # queued ahead of these in-order) have been emitted
            for tc in range(8):
                backg.append((
                    (lambda q2, t2: lambda: oproj_unit(q2, t2))(qc, tc),
                    4800, si, "po"))

    cur_si = len(stretches)
    in_drain[0] = True
    while urgent or backg:
        pump(10000)


def _build():
    global _CACHED_NC
    if _CACHED_NC is not None:
        return _CACHED_NC
    nc = bacc.Bacc("TRN2", target_bir_lowering=False, debug=False)
    xt = nc.dram_tensor("xt", [P, ET * S], bf16, kind="ExternalInput").ap()
    wq = nc.dram_tensor("wq", [P, ET * NPAIR * P], bf16,
                        kind="ExternalInput").ap()
    wk = nc.dram_tensor("wk", [P, ET * NPAIR * P], bf16,
                        kind="ExternalInput").ap()
    wv = nc.dram_tensor("wv", [P, ET * 512], bf16, kind="ExternalInput").ap()
    wo = nc.dram_tensor("wo", [P, NPAIR * D], bf16, kind="ExternalInput").ap()
    out = nc.dram_tensor("out", [S, D], f32, kind="ExternalOutput").ap()

    with tile.TileContext(nc) as tc_:
        _mha_kernel(tc_, [xt, wq, wk, wv, wo], [out])
    nc.compile()
    _CACHED_NC = nc
    return nc


def kernel(x: np.ndarray, Wq: np.ndarray, Wk: np.ndarray, Wv: np.ndarray,
           Wo: np.ndarray) -> np.ndarray:
    global LAST_EXEC_TIME_NS
    nc = _build()
    bf = ml_dtypes.bfloat16

    x = np.asarray(x, dtype=np.float32)
    Wq = np.asarray(Wq, np.float32)
    Wk = np.asarray(Wk, np.float32)
    Wv = np.asarray(Wv, np.float32)
    Wo = np.asarray(Wo, np.float32)

    in_maps = []
    for c in range(NCORES):
        b, tp = c // 2, c % 2
        hs = tp * 512
        # xt: [D, S] -> [et, 128, S] -> [128, et*S]
        xt = np.ascontiguousarray(
            x[b].T.reshape(ET, P, S).transpose(1, 0, 2)).astype(bf)
        # wq/wk: W[hs:hs+512, :].T = [e, hd] -> [et, 128, pair, 128] -> p-first
        wq = np.ascontiguousarray(
            Wq[hs:hs + 512, :].T.reshape(ET, P, NPAIR, P)
            .transpose(1, 0, 2, 3)).astype(bf)
        wk = np.ascontiguousarray(
            Wk[hs:hs + 512, :].T.reshape(ET, P, NPAIR, P)
            .transpose(1, 0, 2, 3)).astype(bf)
        wv = np.ascontiguousarray(
            Wv[hs:hs + 512, :].T.reshape(ET, P, 512)
            .transpose(1, 0, 2)).astype(bf)
        # wo: Wo[:, hs:hs+512].T = [hd, e] -> [hdtile, 128, 1024] -> p-first
        wo = np.ascontiguousarray(
            Wo[:, hs:hs + 512].T.reshape(NPAIR, P, D)
            .transpose(1, 0, 2)).astype(bf)
        in_maps.append({
            "xt": xt.reshape(P, ET * S),
            "wq": wq.reshape(P, ET * NPAIR * P),
            "wk": wk.reshape(P, ET * NPAIR * P),
            "wv": wv.reshape(P, ET * 512),
            "wo": wo.reshape(P, NPAIR * D),
        })

    trace = bool(os.environ.get("BASS_TRACE"))
    res = run_bass_kernel_spmd(nc, in_maps, core_ids=list(range(NCORES)),
                               trace=trace)
    LAST_EXEC_TIME_NS = res.exec_time_ns

    outs = [np.asarray(r["out"], np.float32) for r in res.results]
    return np.stack([outs[2 * b] + outs[2 * b + 1] for b in range(B)])
